# revision 7
# baseline (speedup 1.0000x reference)
"""CapsNet dynamic-routing kernel for 8 TRN2 NeuronCores.

Problem: x [256,1152,8], W [1152,10,8,16], 3 routing iterations, out [256,10,16,1].

Strategy v2 (replicated first iteration, I-sharded rest):
  Iteration 1 uses uniform coupling c=1/O, so s1 = x_flat @ W_flat needs no
  routing state: every core computes the FULL s1 redundantly (144 matmuls,
  ~23us) instead of AllReduce-ing partials.  The first collective's ncfw
  wake latency is a fixed ~43-65us from execution start regardless of
  doorbell time, so this replicated compute (and the 7.7MB xT/W load it
  needs) rides entirely inside the dead window and AllReduce #1 disappears
  from the critical path.
  After v1 = squash(s1), the agreement/routing chain is I-sharded exactly
  like v1 of this kernel: G = xf_local^T @ v (dense matmul, K=batch),
  agree local, b/c local, s2_partial local (K=local 1152), one AllReduce
  of s2 (82KB bf16), then iteration 3 the same with a final ReduceScatter.
  Host-side tile permutation (roll by 9*rank) puts each core's local
  (i,d)-tiles at positions [0:9] of the full 72-tile xT/Wb tensors, so the
  SPMD program needs no rank arithmetic.
  Matmul operands bf16 (fp32 matmul runs 2 passes at 1/4 rate); PSUM
  accumulation fp32; collectives carry bf16.  Softmax skips
  max-subtraction: |b| stays O(1) for this routing.
"""

import numpy as np

B, I, O, DIN, DOUT = 256, 1152, 10, 8, 16
NCORES = 8
I_SH = I // NCORES          # 144 input capsules per core
ID = I_SH * DIN             # 1152 local (i,d) rows
NT = ID // 128              # 9 local partition tiles of (i,d)
NTF = (I * DIN) // 128      # 72 full partition tiles of (i,d)
BT = B // 128               # 2 partition tiles of batch
OE = O * DOUT               # 160
ROUTING_ITERS = 3
PSH = 128 // NCORES         # 16 output partitions per core (ReduceScatter)
LCH = 12                    # setup DMA chunk size (tiles) for xT/Wb loads

_CACHE = {}


def _bc(ap_mod, ap, n):
    """View an AP with an extra innermost broadcast axis of length n."""
    return ap_mod.AP(tensor=ap.tensor, offset=ap.offset, ap=[*ap.ap, [0, n]])


def _build():
    import concourse.bass as bass
    import concourse.bacc as bacc
    import concourse.tile as tile
    from concourse import mybir

    f32 = mybir.dt.float32
    bf16 = mybir.dt.bfloat16
    AF = mybir.ActivationFunctionType
    ALU = mybir.AluOpType

    nc = bacc.Bacc("TRN2", target_bir_lowering=False, debug=False,
                   num_devices=NCORES)

    # All inputs pre-tiled on host to [128, ...] so every DMA is contiguous.
    # xT/Wb are FULL (i,d)-tiled tensors, tile-rolled per core so the local
    # I-shard is tiles [0:NT].
    xT_d = nc.dram_tensor("xT", [128, NTF, B], bf16, kind="ExternalInput")
    xf_d = nc.dram_tensor("xf", [128, BT, ID], bf16, kind="ExternalInput")
    Wb_d = nc.dram_tensor("Wb", [128, NTF, OE], bf16, kind="ExternalInput")
    MB_d = nc.dram_tensor("Mblk", [128, 128], bf16, kind="ExternalInput")
    # final iteration uses ReduceScatter: each core emits 16 partitions
    # x [BT, OE] (batch rows bt*128 + 16*rank + p)
    out_d = nc.dram_tensor("out", [PSH, BT, OE], f32, kind="ExternalOutput")

    with tile.TileContext(nc) as tc:
        with (
            tc.tile_pool(name="sb", bufs=1) as sb,
            tc.tile_pool(name="work", bufs=2) as work,
            tc.tile_pool(name="ps_s", bufs=2, space="PSUM") as ps_s,
            tc.tile_pool(name="ps_g", bufs=2, space="PSUM") as ps_g,
            tc.tile_pool(name="ps_a", bufs=2, space="PSUM") as ps_a,
            tc.tile_pool(name="dram", bufs=3, space="DRAM") as dram,
        ):
            # ---- persistent SBUF tensors ----
            xT = sb.tile([128, NTF, B], bf16)     # full x_flat^T (lhsT for s)
            xf = sb.tile([128, BT, ID], bf16)     # local x_flat (lhsT for G)
            Wb = sb.tile([128, NTF, OE], bf16)    # full W_flat bf16
            Mblk = sb.tile([128, 128], bf16)      # 8x8 block-diag ones
            bq = sb.tile([128, NT, O], f32)       # local routing logits b
            Wc = sb.tile([128, NT, OE], bf16)     # local c * W
            s_sb = sb.tile([128, BT, OE], bf16)   # local partial s (iters 2+)
            sf = sb.tile([128, BT, OE], bf16)     # all-reduced s (iters 2+)
            vb = sb.tile([128, BT, OE], bf16)     # squash(s) bf16 (rhs for G)
            WG = sb.tile([128, NT, OE], bf16)     # W * G (local)
            A1 = sb.tile([128, NT, O], bf16)      # e-reduced agreement
            dmy = sb.tile([128, 1], f32)          # ACT table-prefetch scratch

            # Setup loads, chunked + spread over the three DMA-capable
            # engines so the 7.7MB xT/Wb stream keeps pace with the s1
            # matmuls that consume it in k-order.
            nc.gpsimd.dma_start(out=Mblk[:], in_=MB_d[:])
            for c0 in range(0, NTF, LCH):
                c1 = min(c0 + LCH, NTF)
                nc.scalar.dma_start(out=Wb[:, c0:c1], in_=Wb_d[:, c0:c1])
                nc.sync.dma_start(out=xT[:, c0:c1], in_=xT_d[:, c0:c1])
            nc.gpsimd.dma_start(out=xf[:], in_=xf_d[:])

            # Pre-load the Sqrt ACT table during setup (table loads are
            # ~1.5us each and otherwise land on the critical chain).
            nc.scalar.activation(out=dmy[:], in_=Mblk[:, 0:1], func=AF.Sqrt)

            Wb4 = Wb.rearrange("p t (o e) -> p t o e", o=O)
            Wc4 = Wc.rearrange("p t (o e) -> p t o e", o=O)
            WG4 = WG.rearrange("p t (o e) -> p t o e", o=O)
            sf4 = sf.rearrange("p b (o e) -> p b o e", o=O)
            vb4 = vb.rearrange("p b (o e) -> p b o e", o=O)

            s1_ps = [None, None]  # iter-1 full-s PSUM tiles, one per bt

            for it in range(ROUTING_ITERS):
                first, last = it == 0, it == ROUTING_ITERS - 1

                if first:
                    # s1 = x_flat @ W_flat over the FULL K=9216 (c=1/O folded
                    # into the squash scale).  Replicated on every core; no
                    # collective.  k-major loop so each freshly-DMA'd chunk
                    # is consumed once for both batch tiles.
                    for bt in range(BT):
                        s1_ps[bt] = ps_s.tile([128, OE], f32,
                                              name=f"s1_{bt}", tag="s_ps")
                    for k in range(NTF):
                        for bt in range(BT):
                            nc.tensor.matmul(
                                s1_ps[bt][:],
                                xT[:, k, bt * 128:(bt + 1) * 128],
                                Wb[:, k, :],
                                start=(k == 0), stop=(k == NTF - 1))
                    # stage to SBUF f32: squash reads SBUF, not PSUM
                    sf1 = sb.tile([128, BT, OE], f32)
                    for bt in range(BT):
                        nc.vector.tensor_copy(sf1[:, bt, :], s1_ps[bt][:])
                else:
                    # c = softmax(b) over o per local (i,d) row; |b| is O(1)
                    # so no max-subtraction is needed.
                    ex = work.tile([128, NT, O], f32, tag="ex")
                    nc.scalar.activation(out=ex[:], in_=bq[:], func=AF.Exp)
                    # prefetch Sqrt table for this iteration's squash; rides
                    # the s-matmul + AllReduce slack
                    nc.scalar.activation(out=dmy[:], in_=ex[:, 0, 0:1],
                                         func=AF.Sqrt)
                    sm = work.tile([128, NT], f32, tag="sm")
                    nc.vector.reduce_sum(out=sm[:], in_=ex[:],
                                         axis=mybir.AxisListType.X)
                    nc.vector.reciprocal(out=sm[:], in_=sm[:])
                    nc.vector.tensor_tensor(
                        out=ex[:], in0=ex[:], in1=_bc(bass, sm[:], O),
                        op=ALU.mult)
                    # Wc = c * W (local tiles only) in chunks so the first
                    # s-matmuls start while later tiles still build
                    GRP = 3
                    for g in range(0, NT, GRP):
                        nc.vector.tensor_tensor(
                            out=Wc4[:, g:g + GRP],
                            in0=_bc(bass, ex[:, g:g + GRP, :], DOUT),
                            in1=Wb4[:, g:g + GRP], op=ALU.mult)

                    # s_partial = x_local @ Wc : out [b-tile 128, OE].
                    cc_in = dram.tile([128, BT, OE], bf16, tag="cc_in")
                    for bt in range(BT):
                        s_ps = ps_s.tile([128, OE], f32, tag="s_ps")
                        for k in range(NT):
                            nc.tensor.matmul(
                                s_ps[:],
                                xT[:, k, bt * 128:(bt + 1) * 128],
                                Wc[:, k, :],
                                start=(k == 0), stop=(k == NT - 1))
                        nc.vector.tensor_copy(s_sb[:, bt, :], s_ps[:])
                        nc.sync.dma_start(out=cc_in[:, bt, :],
                                          in_=s_sb[:, bt, :])

                    if last:
                        # Final iteration: each core only needs 1/8 of v, so
                        # ReduceScatter; the shard is 16 partitions x
                        # [BT, OE] (batch rows bt*128 + 16*rank + p).
                        cc_rs = dram.tile([PSH, BT, OE], bf16, tag="cc_rs")
                        nc.gpsimd.collective_compute(
                            "ReduceScatter", ALU.add,
                            replica_groups=[list(range(NCORES))],
                            ins=[cc_in.opt()], outs=[cc_rs.opt()])
                        s3 = sb.tile([PSH, BT, OE], bf16)
                        nc.sync.dma_start(out=s3[:], in_=cc_rs[:])
                        sq3 = work.tile([PSH, BT, OE], f32, tag="sq3")
                        nc.vector.tensor_tensor(out=sq3[:], in0=s3[:],
                                                in1=s3[:], op=ALU.mult)
                        ss3 = work.tile([PSH, BT, O], f32, tag="ss3")
                        nc.vector.reduce_sum(
                            out=ss3[:],
                            in_=sq3.rearrange("p b (o e) -> p b o e", o=O),
                            axis=mybir.AxisListType.X)
                        t13 = work.tile([PSH, BT, O], f32, tag="t13")
                        nc.scalar.activation(out=t13[:], in_=ss3[:],
                                             func=AF.Sqrt)
                        den3 = work.tile([PSH, BT, O], f32, tag="den3")
                        nc.vector.tensor_scalar_add(den3[:], ss3[:], 1.0)
                        nc.vector.reciprocal(out=den3[:], in_=den3[:])
                        rat3 = work.tile([PSH, BT, O], f32, tag="rat3")
                        nc.vector.tensor_tensor(out=rat3[:], in0=t13[:],
                                                in1=den3[:], op=ALU.mult)
                        v3 = work.tile([PSH, BT, OE], f32, tag="v3")
                        nc.vector.tensor_tensor(
                            out=v3.rearrange("p b (o e) -> p b o e", o=O),
                            in0=s3.rearrange("p b (o e) -> p b o e", o=O),
                            in1=_bc(bass, rat3[:], DOUT), op=ALU.mult)
                        nc.sync.dma_start(out=out_d[:], in_=v3[:])
                        continue

                    # AllReduce s over the 8 I-shards
                    cc_out = dram.tile([128, BT, OE], bf16, tag="cc_out",
                                       addr_space="Shared")
                    nc.gpsimd.collective_compute(
                        "AllReduce", ALU.add,
                        replica_groups=[list(range(NCORES))],
                        ins=[cc_in.opt()], outs=[cc_out.opt()])
                    nc.sync.dma_start(out=sf[:], in_=cc_out[:])

                # squash: v = s * sqrt(ss)/(1+ss) per (b, o); iteration 1
                # reads s from PSUM (f32) and carries c=1/O as
                # s_raw = O*s_true.
                if first:
                    sin = [sf1[:, bt, :].rearrange("p (o e) -> p o e", o=O)
                           for bt in range(BT)]
                else:
                    sin = [sf4[:, bt] for bt in range(BT)]
                sq = work.tile([128, BT, OE], f32, tag="sq")
                sq4 = sq.rearrange("p b (o e) -> p b o e", o=O)
                for bt in range(BT):
                    nc.vector.tensor_tensor(out=sq4[:, bt], in0=sin[bt],
                                            in1=sin[bt], op=ALU.mult)
                ss = work.tile([128, BT, O], f32, tag="ss")
                nc.vector.reduce_sum(
                    out=ss[:], in_=sq4,
                    axis=mybir.AxisListType.X)
                t1 = work.tile([128, BT, O], f32, tag="t1")
                nc.scalar.activation(out=t1[:], in_=ss[:], func=AF.Sqrt)
                den = work.tile([128, BT, O], f32, tag="den")
                if first:
                    # ss_raw = O^2*ss_true:
                    #   v = s_raw*(1/O^2)*sqrt(ss_raw)/(1+ss_raw/O^2)
                    nc.vector.tensor_scalar(
                        out=den[:], in0=ss[:], scalar1=1.0 / (O * O),
                        scalar2=1.0, op0=ALU.mult, op1=ALU.add)
                else:
                    nc.vector.tensor_scalar_add(den[:], ss[:], 1.0)
                nc.vector.reciprocal(out=den[:], in_=den[:])
                rat = work.tile([128, BT, O], f32, tag="rat")
                nc.vector.tensor_tensor(out=rat[:], in0=t1[:], in1=den[:],
                                        op=ALU.mult)
                if first:
                    nc.vector.tensor_scalar_mul(rat[:], rat[:], 1.0 / (O * O))
                # prefetch Exp table for the next softmax; rides the
                # agreement-path slack
                nc.scalar.activation(out=dmy[:], in_=rat[:, 0, 0:1],
                                     func=AF.Exp)
                for bt in range(BT):
                    nc.vector.tensor_tensor(
                        out=vb4[:, bt], in0=sin[bt],
                        in1=_bc(bass, rat[:, bt], DOUT), op=ALU.mult)

                # G = xf_local^T @ v ; agree = (1/B) sum_de W*G ; b += agree.
                # Three (i,d)-tiles share one PSUM bank (3*640B < 2KB) so
                # the W*G multiply and e-reduction run once per group.
                GW = 3
                for g in range(0, NT, GW):
                    g_ps = ps_g.tile([128, GW, OE], f32, tag="g_ps")
                    for j in range(GW):
                        for bt in range(BT):
                            nc.tensor.matmul(
                                g_ps[:, j, :],
                                xf[:, bt, (g + j) * 128:(g + j + 1) * 128],
                                vb[:, bt, :],
                                start=(bt == 0), stop=(bt == BT - 1))
                    nc.vector.tensor_tensor(
                        out=WG[:, g:g + GW, :], in0=Wb[:, g:g + GW, :],
                        in1=g_ps[:], op=ALU.mult)
                    with nc.allow_low_precision("agreement tolerates bf16"):
                        nc.vector.reduce_sum(
                            out=A1[:, g:g + GW, :],
                            in_=WG[:, g:g + GW, :].rearrange(
                                "p g (o e) -> p (g o) e", o=O),
                            axis=mybir.AxisListType.X)
                # d-sums of all NT tiles land in one PSUM tile so the
                # b-update is a single vector op
                a_ps = ps_a.tile([128, NT, O], f32, tag="a_ps")
                for mt in range(NT):
                    nc.tensor.matmul(a_ps[:, mt, :], Mblk[:],
                                     A1[:, mt, :], start=True, stop=True)
                if first:
                    nc.vector.tensor_scalar_mul(bq[:], a_ps[:], 1.0 / B)
                else:
                    nc.vector.scalar_tensor_tensor(
                        out=bq[:], in0=a_ps[:], scalar=1.0 / B,
                        in1=bq[:], op0=ALU.mult, op1=ALU.add)

    nc.compile()
    return nc


def _get_nc():
    if "nc" not in _CACHE:
        _CACHE["nc"] = _build()
    return _CACHE["nc"]


def _tile128(a):
    """[R, C] -> [128, R//128, C] with row r = t*128+p at [p, t]."""
    r, c = a.shape
    return np.ascontiguousarray(
        a.reshape(r // 128, 128, c).transpose(1, 0, 2))


def _make_in_maps(x, W):
    from concourse import mybir
    bfdt = mybir.dt.np(mybir.dt.bfloat16)
    x = np.asarray(x, dtype=np.float32)
    W = np.asarray(W, dtype=np.float32)
    mblk = np.kron(np.eye(16, dtype=np.float32),
                   np.ones((8, 8), dtype=np.float32)).astype(bfdt)
    # full flattened operands, in (i,d)-tile granularity for the roll
    x_flat = x.reshape(B, I * DIN)                       # [256, 9216]
    w_flat = W.transpose(0, 2, 1, 3).reshape(I * DIN, OE)  # [9216, 160]
    xT_t = np.ascontiguousarray(x_flat.T).reshape(NTF, 128, B)
    wb_t = w_flat.reshape(NTF, 128, OE)
    in_maps = []
    for core in range(NCORES):
        roll = np.roll(np.arange(NTF), -NT * core)
        xT_c = xT_t[roll].reshape(I * DIN, B)
        wb_c = wb_t[roll].reshape(I * DIN, OE)
        isl = slice(core * I_SH, (core + 1) * I_SH)
        xf_c = x[:, isl, :].reshape(B, ID)
        in_maps.append({
            "xT": _tile128(xT_c).astype(bfdt),
            "xf": _tile128(xf_c).astype(bfdt),
            "Wb": _tile128(wb_c).astype(bfdt),
            "Mblk": mblk,
        })
    return in_maps


def _ensure_ntff_hook():
    """This image's antenv lacks axon_hooks; reconstruct it so trace=True
    can reach the NTFF profiler in libaxon_pjrt.so."""
    import sys
    import types
    try:
        import antenv.axon_hooks  # noqa: F401
        return
    except ImportError:
        pass
    try:
        import antenv
        from trn_agent_boot.trn_boot import _ntff_profile_via_ctypes
        hook = _ntff_profile_via_ctypes("/opt/axon/libaxon_pjrt.so")
        mod = types.ModuleType("antenv.axon_hooks")
        mod._hook = hook
        mod.get_axon_ntff_profile_hook = lambda: mod._hook
        mod.set_axon_ntff_profile_hook = (
            lambda h: setattr(mod, "_hook", h))
        sys.modules["antenv.axon_hooks"] = mod
        antenv.axon_hooks = mod
    except Exception as e:  # profiling is best-effort
        print("ntff hook setup failed:", e)


def _run_hw(x, W, trace=False, **kwargs):
    from concourse import bass_utils
    if trace:
        _ensure_ntff_hook()
    nc = _get_nc()
    res = bass_utils.run_bass_kernel_spmd(
        nc, _make_in_maps(x, W), core_ids=list(range(NCORES)),
        trace=trace, **kwargs)
    shards = np.stack([res.results[c]["out"] for c in range(NCORES)])
    return _assemble(shards), res


def _assemble(shards):
    """shards [NCORES, 16, BT, OE] -> full [B, O, DOUT, 1]; core r's shard
    holds batch rows bt*128 + 16*r + p (ReduceScatter partition sharding)."""
    shards = np.asarray(shards, dtype=np.float32).reshape(
        NCORES, 128 // NCORES, BT, OE)
    # [r, p, bt, f] -> [bt, r, p, f] -> [B, OE]
    full = shards.transpose(2, 0, 1, 3).reshape(B, OE)
    return full.reshape(B, O, DOUT)[..., None]


def kernel(x, W):
    out, _ = _run_hw(x, W, trace=False)
    return out
